# revision 22
# baseline (speedup 1.0000x reference)
"""Batched GNN neighbor aggregation on 8 NeuronCores.

out[b] = neibors[b] @ last_embs[b]  for b in 0..7  (2048x2048 @ 2048x128, f32)

Sharding: one graph per core (batch dim across the 8 cores), no cross-core
communication. The PE contracts over the partition dimension, so the
adjacency operand sits in SBUF with the contraction index (m) on
partitions; each graph's adjacency is pre-transposed on the host during
sharding so the device streams it with fully-contiguous DMAs.

Precision: the rel-err budget is 2e-2 (absmax / max|expected|); a single
bf16 pass for both operands with f32 PSUM accumulation and a bf16 output
measures 3.2e-3 on the real data — 6x margin. (fp8e4m3 single-pass
adjacency measures 2.49e-2 and fails; DoubleRow is fp8-only, so bf16
1 col/cycle is the PE floor.) Stream per core, 9.0 MiB total:
  a_hi  16 x [128 x 2048] bf16 chunks   8.0 MiB
  e     [128, 16, 128] bf16             0.5 MiB
  out_t [128, 2048] bf16 (transposed)   0.5 MiB
The device computes out^T = embs^T @ neibors^T with the embedding k-chunks
stationary; the host transposes the small result back and upcasts to f32.

Steady state is HBM-bound: 8 cores x 9.44 MB against the ~2.9 TB/s chip
HBM gives a ~26 us chip-level streaming floor; measured exec adds ~7 us
of runtime/engine-boot preamble and ~6 us of drain + runtime-epilogue.
Chunk DMAs alternate between the two HWDGE rings (sync + scalar) with a
16-deep tile pool so all transfers queue immediately. A short scratch-
matmul pre-warm starts the PE DVFS ramp during the DMA preamble. DMA
completion latency scales with transfer size, so chunk 0 and the head
of e are split small to unblock the first real matmul ~1.5 us earlier.
The last chunk lands per n-block split across both rings so the final
matmul -> cast -> store drain pipelines bank-by-bank, and the last
bank's cast/store is halved across DVE/ACT and both rings.
"""

import numpy as np
import ml_dtypes

BF16 = ml_dtypes.bfloat16

B = 8
N = 2048
D = 128
KT = 128
NT = 512
NK = N // KT   # 16
NN = N // NT   # 4

_cached_nc = None


def _dedup_ldweights(nc, mybir):
    """Drop InstLdweights whose weight AP matches the immediately preceding
    weight load in the PE stream (matmuls here have ldweights=False, so the
    stationary operand stays in the array between identical loads)."""
    for bb in nc.m.functions[0].blocks:
        insts = bb.instructions
        last_key = None
        removed = []
        for inst in insts:
            if getattr(inst, "engine", None) != mybir.EngineType.PE:
                continue
            ty = type(inst).__name__
            if ty == "InstLdweights":
                key = repr(inst.ins[0])
                if key == last_key and not inst.has_wait():
                    removed.append(inst)
                else:
                    last_key = key
            elif ty != "InstMatmult":
                last_key = None
        if removed:
            rm = {id(i) for i in removed}
            insts[:] = [i for i in insts if id(i) not in rm]
            for i in removed:
                nc.inst_map.pop(i.name, None)


def _build_program():
    import concourse.tile as tile
    from concourse import bacc, mybir

    f32 = mybir.dt.float32
    bf16 = mybir.dt.bfloat16
    nc = bacc.Bacc(
        "TRN2",
        target_bir_lowering=False,
        debug=False,
        enable_asserts=False,
        enable_partition_id=False,
    )

    a_hi = nc.dram_tensor("a_hi", [NK, KT, N], bf16, kind="ExternalInput")
    # e[p, k, d] = E[k*KT + p, d]
    e = nc.dram_tensor("e", [KT, NK, D], bf16, kind="ExternalInput")
    out_t = nc.dram_tensor("out_t", [D, N], bf16, kind="ExternalOutput")

    with tile.TileContext(nc) as tc:
        with (
            tc.tile_pool(name="econst", bufs=1) as epool,
            tc.tile_pool(name="ahi", bufs=16) as hpool,
            tc.tile_pool(name="psum", bufs=1, space="PSUM") as pspool,
            tc.tile_pool(name="out", bufs=1) as opool,
        ):
            # Short HAM pre-warm: start the PE clock ramp during the DMA
            # preamble without delaying the first real matmul.
            wu = epool.tile([KT, KT], bf16, name="wu")
            wu_ps = pspool.tile([KT, KT], f32, name="wups", tag="wups")
            nc.vector.memset(wu[:], 0.0)
            for _ in range(6):
                nc.tensor.matmul(wu_ps[:], wu[:], wu[:], start=True, stop=True)

            # split so the first matmuls aren't gated on full 512 KB
            # DMA completions (completion latency scales with size);
            # e head (chunks 0-3) first on scalar, tail on sync where
            # there is early slack — balances both rings at ~4.5 MiB
            e_sb = epool.tile([KT, NK, D], bf16, name="e_sb")
            nc.scalar.dma_start(e_sb[:, :4, :], e.ap()[:, :4, :])

            ps = [
                pspool.tile([D, NT], f32, name=f"ps{n}", tag=f"ps{n}")
                for n in range(NN)
            ]

            for k in range(NK):
                hi = hpool.tile([KT, N], bf16, tag="hi")
                eng = nc.sync if k % 2 == 0 else nc.scalar
                if k == NK - 1:
                    # last chunk: land per n-block, split across both
                    # rings, so the final drain pipelines bank-by-bank
                    for n in range(NN):
                        (nc.sync if n % 2 == 0 else nc.scalar).dma_start(
                            hi[:, n * NT : (n + 1) * NT],
                            a_hi.ap()[k][:, n * NT : (n + 1) * NT],
                        )
                elif k == 0:
                    # quarters alternating rings: each 128 KB piece
                    # completes fast and both rings work immediately
                    for n in range(NN):
                        (nc.sync if n % 2 == 0 else nc.scalar).dma_start(
                            hi[:, n * NT : (n + 1) * NT],
                            a_hi.ap()[k][:, n * NT : (n + 1) * NT],
                        )
                    # e tail (chunks 4-15) on sync behind chunk 0's
                    # quarters; first needed by the chunk-4 matmuls
                    nc.sync.dma_start(e_sb[:, 4:, :], e.ap()[:, 4:, :])
                else:
                    eng.dma_start(hi[:], a_hi.ap()[k])

                if k < NK - 1:
                    for n in range(NN):
                        nc.tensor.matmul(
                            ps[n][:],
                            e_sb[:, k, :],
                            hi[:, n * NT : (n + 1) * NT],
                            start=(k == 0),
                            stop=False,
                        )
                else:
                    # bank-major: copy + store of bank n overlap the
                    # matmul of bank n+1; casts alternate DVE/ACT so they
                    # pipeline too
                    for n in range(NN):
                        nc.tensor.matmul(
                            ps[n][:],
                            e_sb[:, k, :],
                            hi[:, n * NT : (n + 1) * NT],
                            start=False,
                            stop=True,
                        )
                        o_sb = opool.tile([D, NT], bf16, name=f"o{n}", tag=f"o{n}")
                        if n < NN - 1:
                            if n % 2 == 0:
                                nc.vector.tensor_copy(o_sb[:], ps[n][:])
                            else:
                                nc.scalar.copy(o_sb[:], ps[n][:])
                            (nc.sync if n % 2 == 0 else nc.scalar).dma_start(
                                out_t.ap()[:, n * NT : (n + 1) * NT], o_sb[:]
                            )
                        else:
                            # last bank: halve the cast+store across both
                            # engines/rings to shorten the final chain
                            h = NT // 2
                            nc.vector.tensor_copy(o_sb[:, :h], ps[n][:, :h])
                            nc.scalar.copy(o_sb[:, h:], ps[n][:, h:])
                            nc.sync.dma_start(
                                out_t.ap()[:, n * NT : n * NT + h], o_sb[:, :h]
                            )
                            nc.scalar.dma_start(
                                out_t.ap()[:, n * NT + h : (n + 1) * NT],
                                o_sb[:, h:],
                            )

    try:
        _dedup_ldweights(nc, mybir)
    except Exception:
        pass
    nc.compile()
    return nc


def _make_in_maps(last_embs, neibors):
    in_maps = []
    for g in range(B):
        at_g = np.ascontiguousarray(neibors[g].T)  # [m, n] f32
        ah = at_g.astype(BF16)
        eg = np.asarray(last_embs[g]).astype(BF16)  # [N, D]
        e = np.ascontiguousarray(eg.reshape(NK, KT, D).transpose(1, 0, 2))
        in_maps.append(
            {
                "a_hi": np.ascontiguousarray(ah.reshape(NK, KT, N)),
                "e": e,
            }
        )
    return in_maps


def kernel(last_embs, neibors):
    global _cached_nc
    from concourse.bass_utils import run_bass_kernel_spmd

    last_embs = np.asarray(last_embs, dtype=np.float32)
    neibors = np.asarray(neibors, dtype=np.float32)
    if _cached_nc is None:
        _cached_nc = _build_program()
    in_maps = _make_in_maps(last_embs, neibors)
    try:
        res = run_bass_kernel_spmd(_cached_nc, in_maps, list(range(B))).results
    except Exception:
        # transient NRT/terminal hiccups have been observed; retry once
        import time

        time.sleep(15)
        res = run_bass_kernel_spmd(_cached_nc, in_maps, list(range(B))).results
    out = np.stack(
        [res[g]["out_t"].T.astype(np.float32) for g in range(B)], axis=0
    )
    return np.ascontiguousarray(out)
